# revision 5
# baseline (speedup 1.0000x reference)
"""GATv2Conv Trainium2 kernel (8 NeuronCores, SPMD, no collectives).

Strategy
--------
Shard target nodes across the 8 cores (2560 nodes each, node-padded to
20480).  Every edge lives on the core that owns its dst node, so the
segment-softmax and the weighted aggregation are core-local: no
cross-core collective is needed.  Each core's edges are grouped by
128-node block (20 blocks/core) and padded to a fixed 9 tiles of 128
edges per block, so the SPMD program is identical on all cores.

Following the sharding hint ("shard edges and their gathered endpoint
features per device"), the host gathers each edge's endpoint rows while
sharding: per core it ships pre-transposed x[src] and edge_attr streams
(lhsT layout for the TensorEngine), an untransposed edge_attr stream
(for the self-loop mean), and 0/1 edge->node indicator tiles S / S_T
built from dst (padding edges get all-zero rows, neutralizing them).
This is pure index shuffling - every FLOP stays on device.

Per edge tile [128 edges] the device:
  - computes s = x_src@Wl + ea@We + S@XR_block as one PSUM accumulation
    chain (XR_block is projected once per block), plus xl = x_src@Wl
    alone in a second bank for the aggregation;
  - logits = att . LeakyReLU(s) (ACT Prelu + DVE mult/grouped-reduce);
    p = exp(logits) (softmax max-subtraction is skipped: logits are
    O(+-10) so exp is safe in fp32, and softmax is shift-invariant);
  - accumulates loop_attr sums += S.T@ea and denom += S.T@p into one
    PSUM bank (single bank-clear on the block's first matmul - PE
    `start` clears the whole bank), and out_unnorm += S.T@(p (x) xl)
    into another.
Normalization commutes with the segment sum (all edges of a node share
its denominator), so alpha is never materialized per edge:
out = inv_denom (x) out_unnorm + alpha_loop (x) xl_n + bias, computed
densely per node block (self loops use the fill_value='mean' attr from
the accumulated sums; in-degree reciprocals ship from the host since
they are index-only data).
"""

import os
import sys

for _p in ("/opt/trn_rl_repo",):
    if _p not in sys.path and os.path.isdir(_p):
        sys.path.insert(0, _p)

import numpy as np
import ml_dtypes

import concourse.bacc as bacc
import concourse.mybir as mybir
import concourse.tile as tile
from concourse.bass_utils import run_bass_kernel_spmd
from concourse.masks import make_identity

# Problem shape (hardcoded per contract)
N = 20000       # nodes
E = 160000      # edges (before self loops)
IN = 128        # in_channels == edge_dim
H = 8           # heads
C = 64          # channels/head
HC = H * C      # 512
NEG = 0.2       # leaky relu slope

NCORES = 8
NPC = 2560      # nodes per core
NPAD = NPC * NCORES
NBLK = 20       # 128-node blocks per core
BN = 128        # nodes per block
TPB = 9         # edge tiles per block (max block in-degree 1114 <= 9*128)
ET = 128        # edges per tile
NT = NBLK * TPB  # edge tiles per core (180)
EC = NT * ET     # padded edge slots per core

BF16 = mybir.dt.bfloat16
F32 = mybir.dt.float32

_CACHE = {}


def _build_program():
    nc = bacc.Bacc("TRN2", target_bir_lowering=False, debug=False,
                   enable_asserts=False, num_devices=NCORES)

    # ---- DRAM parameters (name-keyed in in_maps) ----
    xsrcT_d = nc.declare_dram_parameter("xsrcT", [128, EC], BF16, isOutput=False)
    eaT_d = nc.declare_dram_parameter("eaT", [128, EC], BF16, isOutput=False)
    ear_d = nc.declare_dram_parameter("ear", [128, NT * IN], BF16, isOutput=False)
    s_d = nc.declare_dram_parameter("s_mat", [128, NT * BN], BF16, isOutput=False)
    st_d = nc.declare_dram_parameter("st_mat", [128, NT * ET], BF16, isOutput=False)
    xownT_d = nc.declare_dram_parameter("xownT", [128, NPC], BF16, isOutput=False)
    wl_d = nc.declare_dram_parameter("wl", [IN, HC], BF16, isOutput=False)
    wr_d = nc.declare_dram_parameter("wr", [IN, HC], BF16, isOutput=False)
    we_d = nc.declare_dram_parameter("we", [IN, HC], BF16, isOutput=False)
    att_d = nc.declare_dram_parameter("att_b", [128, HC], BF16, isOutput=False)
    bias_d = nc.declare_dram_parameter("bias_b", [128, HC], F32, isOutput=False)
    cinv_d = nc.declare_dram_parameter("cinv", [128, NBLK], F32, isOutput=False)
    out_d = nc.declare_dram_parameter("out", [NPC, HC], F32, isOutput=True)

    AL = mybir.AluOpType
    AF = mybir.ActivationFunctionType
    BW = TPB * ET  # block width in edge columns (1152)

    with tile.TileContext(nc) as tc:
        with (
            tc.tile_pool(name="const", bufs=1) as cpool,
            tc.tile_pool(name="blkio", bufs=2) as iopool,
            tc.tile_pool(name="work", bufs=3) as wpool,
            tc.tile_pool(name="blk", bufs=2) as bpool,
            tc.tile_pool(name="psS", bufs=2, space="PSUM") as psS,
            tc.tile_pool(name="psXL", bufs=2, space="PSUM") as psXL,
            tc.tile_pool(name="psStat", bufs=1, space="PSUM") as psStat,
            tc.tile_pool(name="psAgg", bufs=1, space="PSUM") as psAgg,
            tc.tile_pool(name="psTn", bufs=1, space="PSUM") as psTn,
        ):
            # ---- resident constants ----
            wl_s = cpool.tile([IN, HC], BF16, tag="wl")
            wr_s = cpool.tile([IN, HC], BF16, tag="wr")
            we_s = cpool.tile([IN, HC], BF16, tag="we")
            att_s = cpool.tile([128, HC], BF16, tag="att")
            bias_s = cpool.tile([128, HC], F32, tag="bias")
            xot_s = cpool.tile([128, NPC], BF16, tag="xot")
            cinv_s = cpool.tile([128, NBLK], F32, tag="cinv")
            idb_s = cpool.tile([128, 128], BF16, tag="idb")

            nc.sync.dma_start(out=wl_s[:], in_=wl_d[:])
            nc.sync.dma_start(out=wr_s[:], in_=wr_d[:])
            nc.sync.dma_start(out=we_s[:], in_=we_d[:])
            nc.sync.dma_start(out=att_s[:], in_=att_d[:])
            nc.sync.dma_start(out=bias_s[:], in_=bias_d[:])
            nc.sync.dma_start(out=xot_s[:], in_=xownT_d[:])
            nc.sync.dma_start(out=cinv_s[:], in_=cinv_d[:])
            make_identity(nc, idb_s[:])

            for b in range(NBLK):
                r0 = b * BN
                c0 = b * BW
                # ---- per-block streaming loads ----
                xsrcT_b = iopool.tile([128, BW], BF16, tag="xsrcT")
                nc.sync.dma_start(out=xsrcT_b[:], in_=xsrcT_d[:, c0:c0 + BW])
                eaT_b = iopool.tile([128, BW], BF16, tag="eaT")
                nc.sync.dma_start(out=eaT_b[:], in_=eaT_d[:, c0:c0 + BW])
                ear_b = iopool.tile([128, BW], BF16, tag="ear")
                nc.sync.dma_start(out=ear_b[:], in_=ear_d[:, c0:c0 + BW])
                s_b = iopool.tile([128, BW], BF16, tag="s_b")
                nc.sync.dma_start(out=s_b[:], in_=s_d[:, c0:c0 + BW])
                st_b = iopool.tile([128, BW], BF16, tag="st_b")
                nc.sync.dma_start(out=st_b[:], in_=st_d[:, c0:c0 + BW])

                # ---- node phase 1: own-block projections ----
                xbT = xot_s[:, r0:r0 + BN]
                pxr = psXL.tile([BN, HC], F32, tag="pxl")
                nc.tensor.matmul(out=pxr[:], lhsT=xbT, rhs=wr_s[:],
                                 start=True, stop=True)
                xr_s = bpool.tile([BN, HC], BF16, tag="xr")
                nc.scalar.copy(out=xr_s[:], in_=pxr[:])

                pxl = psXL.tile([BN, HC], F32, tag="pxl")
                nc.tensor.matmul(out=pxl[:], lhsT=xbT, rhs=wl_s[:],
                                 start=True, stop=True)
                xln_s = bpool.tile([BN, HC], BF16, tag="xln")
                nc.scalar.copy(out=xln_s[:], in_=pxl[:])

                pstat = psStat.tile([BN, IN + H], F32, tag="pstat")
                pagg = psAgg.tile([BN, HC], F32, tag="pagg")

                # ---- edge pass: 9 tiles of 128 edges ----
                for tt in range(TPB):
                    e0 = tt * ET
                    first = tt == 0
                    last = tt == TPB - 1
                    xsrcT_t = xsrcT_b[:, e0:e0 + ET]
                    eaT_t = eaT_b[:, e0:e0 + ET]
                    ear_t = ear_b[:, tt * IN:tt * IN + IN]
                    s_t = s_b[:, tt * BN:tt * BN + BN]
                    st_t = st_b[:, e0:e0 + ET]

                    # xl (alone) and s = xl + xe + xr
                    pxle = psXL.tile([ET, HC], F32, tag="pxl")
                    nc.tensor.matmul(out=pxle[:], lhsT=xsrcT_t, rhs=wl_s[:],
                                     start=True, stop=True)
                    ps = psS.tile([ET, HC], F32, tag="ps")
                    nc.tensor.matmul(out=ps[:], lhsT=xsrcT_t, rhs=wl_s[:],
                                     start=True, stop=False)
                    nc.tensor.matmul(out=ps[:], lhsT=eaT_t, rhs=we_s[:],
                                     start=False, stop=False)
                    nc.tensor.matmul(out=ps[:], lhsT=st_t, rhs=xr_s[:],
                                     start=False, stop=True)

                    # logits / p
                    m_s = wpool.tile([ET, HC], BF16, tag="m_s")
                    nc.scalar.activation(out=m_s[:], in_=ps[:], func=AF.Prelu,
                                         alpha=NEG)
                    lm = wpool.tile([ET, HC], BF16, tag="lm")
                    nc.vector.tensor_tensor(out=lm[:], in0=m_s[:], in1=att_s[:],
                                            op=AL.mult)
                    logit = wpool.tile([ET, H], F32, tag="logit")
                    nc.vector.tensor_reduce(
                        out=logit[:], in_=lm[:].rearrange("p (h c) -> p h c", c=C),
                        axis=mybir.AxisListType.X, op=AL.add)
                    p_t = wpool.tile([ET, H], BF16, tag="p_t")
                    nc.scalar.activation(out=p_t[:], in_=logit[:], func=AF.Exp)

                    # w = p (x) xl   (bf16)
                    w_s = wpool.tile([ET, HC], BF16, tag="w_s")
                    nc.vector.tensor_tensor(
                        out=w_s[:].rearrange("p (h c) -> p h c", c=C),
                        in0=pxle[:].rearrange("p (h c) -> p h c", c=C),
                        in1=p_t[:].to_broadcast([ET, H, C]),
                        op=AL.mult)

                    # segment accumulations (single bank-clear per block)
                    nc.tensor.matmul(out=pstat[:, 0:IN], lhsT=s_t, rhs=ear_t,
                                     start=first, stop=last)
                    nc.tensor.matmul(out=pstat[:, IN:IN + H], lhsT=s_t,
                                     rhs=p_t[:], start=False, stop=last)
                    nc.tensor.matmul(out=pagg[:], lhsT=s_t, rhs=w_s[:],
                                     start=first, stop=last)

                # ---- node phase 2: self loop + finalize ----
                la_s = bpool.tile([BN, IN], BF16, tag="la_s")
                nc.vector.tensor_scalar(out=la_s[:], in0=pstat[:, 0:IN],
                                        scalar1=cinv_s[:, b:b + 1], scalar2=None,
                                        op0=AL.mult)
                ptl = psTn.tile([128, BN], BF16, tag="ptl")
                nc.tensor.transpose(out=ptl[:], in_=la_s[:], identity=idb_s[:])
                laT = bpool.tile([128, BN], BF16, tag="laT")
                nc.scalar.copy(out=laT[:], in_=ptl[:])

                psl = psS.tile([BN, HC], F32, tag="ps")
                nc.tensor.matmul(out=psl[:], lhsT=xbT, rhs=wl_s[:],
                                 start=True, stop=False)
                nc.tensor.matmul(out=psl[:], lhsT=xbT, rhs=wr_s[:],
                                 start=False, stop=False)
                nc.tensor.matmul(out=psl[:], lhsT=laT[:], rhs=we_s[:],
                                 start=False, stop=True)
                ml_s = bpool.tile([BN, HC], BF16, tag="ml_s")
                nc.scalar.activation(out=ml_s[:], in_=psl[:], func=AF.Prelu,
                                     alpha=NEG)
                lml = bpool.tile([BN, HC], BF16, tag="lml")
                nc.vector.tensor_tensor(out=lml[:], in0=ml_s[:], in1=att_s[:],
                                        op=AL.mult)
                logl = bpool.tile([BN, H], F32, tag="logl")
                nc.vector.tensor_reduce(
                    out=logl[:], in_=lml[:].rearrange("p (h c) -> p h c", c=C),
                    axis=mybir.AxisListType.X, op=AL.add)
                pl_f = bpool.tile([BN, H], F32, tag="pl_f")
                nc.scalar.activation(out=pl_f[:], in_=logl[:], func=AF.Exp)

                den = bpool.tile([BN, H], F32, tag="den")
                nc.vector.tensor_tensor(out=den[:], in0=pstat[:, IN:IN + H],
                                        in1=pl_f[:], op=AL.add)
                dinv = bpool.tile([BN, H], F32, tag="dinv")
                nc.vector.reciprocal(out=dinv[:], in_=den[:])
                alphal = bpool.tile([BN, H], F32, tag="alphal")
                nc.vector.tensor_tensor(out=alphal[:], in0=pl_f[:], in1=dinv[:],
                                        op=AL.mult)

                o1 = bpool.tile([BN, HC], F32, tag="o1")
                nc.vector.tensor_tensor(
                    out=o1[:].rearrange("p (h c) -> p h c", c=C),
                    in0=pagg[:].rearrange("p (h c) -> p h c", c=C),
                    in1=dinv[:].to_broadcast([BN, H, C]),
                    op=AL.mult)
                o2 = bpool.tile([BN, HC], F32, tag="o2")
                nc.vector.tensor_tensor(
                    out=o2[:].rearrange("p (h c) -> p h c", c=C),
                    in0=xln_s[:].rearrange("p (h c) -> p h c", c=C),
                    in1=alphal[:].to_broadcast([BN, H, C]),
                    op=AL.mult)
                o3 = bpool.tile([BN, HC], F32, tag="o3")
                nc.vector.tensor_tensor(out=o3[:], in0=o1[:], in1=o2[:], op=AL.add)
                o4 = bpool.tile([BN, HC], F32, tag="o4")
                nc.vector.tensor_tensor(out=o4[:], in0=o3[:], in1=bias_s[:],
                                        op=AL.add)
                nc.sync.dma_start(out=out_d[r0:r0 + BN, :], in_=o4[:])

    nc.compile()
    return nc


def _preprocess(x, edge_index, edge_attr, Wl, Wr, We, att, bias):
    bf = ml_dtypes.bfloat16
    src = np.asarray(edge_index[0], dtype=np.int64)
    dst = np.asarray(edge_index[1], dtype=np.int64)

    x_bf = np.zeros((NPAD, IN), dtype=bf)
    x_bf[:N] = np.asarray(x).astype(bf)
    ea_bf = np.asarray(edge_attr).astype(bf)
    wl_b = np.asarray(Wl).astype(bf)
    wr_b = np.asarray(Wr).astype(bf)
    we_b = np.asarray(We).astype(bf)
    att_b = np.broadcast_to(np.asarray(att).reshape(1, HC), (128, HC)).astype(bf).copy()
    bias_b = np.broadcast_to(np.asarray(bias, dtype=np.float32).reshape(1, HC),
                             (128, HC)).copy()

    order = np.argsort(dst, kind="stable")
    dst_s = dst[order]
    bounds = np.searchsorted(dst_s, np.arange(0, NPAD + BN, BN))

    # slot tables: [core, partition(edge-in-tile), tile]
    src_cols = np.zeros((NCORES, ET, NT), np.int64)
    perm_cols = np.zeros((NCORES, ET, NT), np.int64)
    dst_cols = np.full((NCORES, ET, NT), -1, np.int64)
    for c in range(NCORES):
        for b in range(NBLK):
            g = c * NBLK + b
            eids = order[bounds[g]:bounds[g + 1]]
            k = len(eids)
            assert k <= TPB * ET, f"block {g} has {k} edges > {TPB * ET}"
            j = np.arange(k)
            tl = b * TPB + j // ET
            pp = j % ET
            src_cols[c, pp, tl] = src[eids]
            perm_cols[c, pp, tl] = eids
            dst_cols[c, pp, tl] = dst[eids] - g * BN

    cnt = np.bincount(dst, minlength=NPAD).astype(np.float32)
    cinv_full = (1.0 / np.maximum(cnt, 1.0)).astype(np.float32)
    n_ids = np.arange(BN)

    in_maps = []
    for c in range(NCORES):
        flat = src_cols[c].T.reshape(-1)            # index = t*128+p
        xsrcT = np.ascontiguousarray(x_bf[flat].T)  # [128, EC]
        flatp = perm_cols[c].T.reshape(-1)
        eaT = np.ascontiguousarray(ea_bf[flatp].T)  # [128, EC]
        ear = np.ascontiguousarray(
            ea_bf[perm_cols[c]].reshape(ET, NT * IN))  # [128, NT*128]
        s_mat = np.ascontiguousarray(
            (dst_cols[c][:, :, None] == n_ids[None, None, :])
            .transpose(0, 1, 2).reshape(ET, NT * BN)).astype(bf)
        st_mat = np.ascontiguousarray(
            (n_ids[:, None, None] == dst_cols[c].transpose(1, 0)[None, :, :])
            .reshape(BN, NT * ET)).astype(bf)
        xownT = np.ascontiguousarray(x_bf[c * NPC:(c + 1) * NPC].T)
        cinv = np.ascontiguousarray(
            cinv_full[c * NPC:(c + 1) * NPC].reshape(NBLK, BN).T)  # [128, NBLK]
        in_maps.append({
            "xsrcT": xsrcT, "eaT": eaT, "ear": ear,
            "s_mat": s_mat, "st_mat": st_mat,
            "xownT": xownT, "cinv": cinv,
            "wl": wl_b, "wr": wr_b, "we": we_b,
            "att_b": att_b, "bias_b": bias_b,
        })
    return in_maps


def run(inputs, trace=False, **spmd_kwargs):
    """Build (cached), preprocess, execute; returns (out, BassKernelResults)."""
    if "nc" not in _CACHE:
        _CACHE["nc"] = _build_program()
    nc = _CACHE["nc"]
    in_maps = _preprocess(**inputs)
    res = run_bass_kernel_spmd(nc, in_maps, list(range(NCORES)), trace=trace,
                               **spmd_kwargs)
    outs = [np.asarray(res.results[c]["out"]) for c in range(NCORES)]
    full = np.concatenate(outs, axis=0)[:N]
    return full, res


def kernel(x, edge_index, edge_attr, Wl, Wr, We, att, bias):
    out, _ = run(dict(x=x, edge_index=edge_index, edge_attr=edge_attr,
                      Wl=Wl, Wr=Wr, We=We, att=att, bias=bias))
    return out
